# revision 2
# baseline (speedup 1.0000x reference)
"""NetVLAD pooling kernel for Trainium2 (8 NeuronCores, data-parallel over B).

Math per token m (of B*T=256):  logits = r @ W.T + b ; a = softmax(logits, -1)
    v = a.T @ r - a.sum(0)[:, None] * centroids          (r: [N=2048, C=64], K=32)

Mapping (per core = 32 tokens):
  - GEMM1 (contract C, bias fused): lhsT = rT tiles [65, 128] where row 64 is
    constant 1.0; rhs = [W.T; b] [65, 32]. One matmul per 128-n chunk writes
    logits+b into one PSUM bank [128, 16, 32] per token.
  - softmax: one EXP (Scalar, PSUM f32 -> SBUF bf16), tensor_reduce over k
    (Vector), reciprocal, one bf16 multiply -> a [128, 16, 32] bf16.
  - GEMM2 (contract N, flipped): lhsT = rn chunk [128, 65] (col 64 = -1),
    rhs = a chunk [128, 32]. out = v.T [65, 32] per token, 16 tokens col-packed
    into one PSUM bank [65, 512]; row 64 = -sum_n(a).
  - No device epilogue: Scalar copies each vt bank to SBUF bf16, DMA to DRAM.
    Host applies v += (-asum) * centroids and transposes to [tok, K, C].
"""

import os
import sys

import numpy as np

sys.path.insert(0, "/opt/trn_rl_repo")

import ml_dtypes  # noqa: E402

import concourse.bass as bass  # noqa: E402
import concourse.tile as tile  # noqa: E402
from concourse import mybir  # noqa: E402
from concourse.bass_utils import run_bass_kernel_spmd  # noqa: E402

B, T, N, C, K = 8, 32, 2048, 64, 32
NCORES = 8
TOK = (B * T) // NCORES  # 32 tokens per core
TPB = 4                  # tokens per DMA batch
NB = TOK // TPB          # 8 batches
NCH = N // 128           # 16 n-chunks per token
GRP = 16                 # tokens per vt PSUM bank
NG = TOK // GRP          # 2 groups
LAG = 3                  # GEMM2 trails GEMM1 by LAG tokens (hides softmax)

BF16 = mybir.dt.bfloat16
F32 = mybir.dt.float32

_CACHE = {}

_NO_SPLIT_TYPES = ("InstEventSemaphore",)


def _split_excess_waits(nc):
    """walrus' setupSyncWait refuses >1 sem wait on (at least) the TT-family
    structs. Hoist extra waits onto standalone InstEventSemaphore ops."""
    for f in nc.m.functions:
        for blk in f.blocks:
            out = []
            changed = False
            for inst in blk.instructions:
                si = getattr(inst, "sync_info", None)
                if (
                    si is not None
                    and si.on_wait
                    and len(si.on_wait) > 1
                    and type(inst).__name__ not in _NO_SPLIT_TYPES
                ):
                    for idx, w in enumerate(si.on_wait[:-1]):
                        out.append(
                            mybir.InstEventSemaphore(
                                name=f"{inst.name}_xw{idx}",
                                engine=inst.engine,
                                sync_info=mybir.SyncInfo(on_wait=[w], on_update=[]),
                            )
                        )
                    inst.sync_info = mybir.SyncInfo(
                        on_wait=[si.on_wait[-1]], on_update=si.on_update
                    )
                    changed = True
                out.append(inst)
            if changed:
                try:
                    blk.instructions[:] = out
                except TypeError:
                    blk.instructions = out


def _build_nc(split_waits=True):
    stage = int(os.environ.get("NETVLAD_STAGE", "3"))  # 1=G1+exp 2=+softmax 3=full
    nc = bass.Bass()
    RT = nc.declare_dram_parameter("RT", [NB, 65, TPB, N], BF16, False)
    RN = nc.declare_dram_parameter("RN", [NB, 128, TPB, NCH, 65], BF16, False)
    WTB = nc.declare_dram_parameter("WTB", [65, K], BF16, False)
    VO = nc.declare_dram_parameter("VO", [NG, 65, GRP * K], BF16, True)

    with tile.TileContext(nc) as tc:
        with (
            tc.tile_pool(name="singles", bufs=1) as singles,
            tc.tile_pool(name="rt", bufs=3) as rt_pool,
            tc.tile_pool(name="rn", bufs=3) as rn_pool,
            tc.tile_pool(name="e", bufs=3) as e_pool,
            tc.tile_pool(name="a", bufs=LAG + 3) as a_pool,
            tc.tile_pool(name="s", bufs=6) as s_pool,
            tc.tile_pool(name="o", bufs=2) as o_pool,
            tc.tile_pool(name="pl", bufs=3, space="PSUM") as pl_pool,
            tc.tile_pool(name="pv", bufs=2, space="PSUM") as pv_pool,
        ):
            wtb_sb = singles.tile([65, K], BF16)
            nc.sync.dma_start(out=wtb_sb[:], in_=WTB[:])

            rt_sb = [None] * NB
            rn_sb = [None] * NB
            pv = [None] * NG
            a_t = [None] * TOK

            def load_batch(bi):
                rt_sb[bi] = rt_pool.tile([65, TPB, N], BF16, name="rt_t", tag="rt_t")
                nc.sync.dma_start(out=rt_sb[bi][:], in_=RT[bi])
                rn_sb[bi] = rn_pool.tile(
                    [128, TPB, NCH, 65], BF16, name="rn_t", tag="rn_t"
                )
                nc.sync.dma_start(out=rn_sb[bi][:], in_=RN[bi])

            def gemm1_softmax(tok):
                bi, ti = tok // TPB, tok % TPB
                pl = pl_pool.tile([128, NCH, K], F32)
                for j in range(NCH):
                    nc.tensor.matmul(
                        pl[:, j, :],
                        rt_sb[bi][0:65, ti, 128 * j : 128 * j + 128],
                        wtb_sb[0:65, :],
                        start=True,
                        stop=True,
                        skip_group_check=True,
                    )
                e = e_pool.tile([128, NCH, K], BF16)
                nc.scalar.activation(
                    e[:], pl[:], mybir.ActivationFunctionType.Exp
                )
                if stage <= 1:
                    a_t[tok] = e
                    return
                s = s_pool.tile([128, NCH], F32)
                nc.vector.tensor_reduce(
                    s[:], e[:], axis=mybir.AxisListType.X, op=mybir.AluOpType.add
                )
                rs = s_pool.tile([128, NCH], F32)
                nc.vector.reciprocal(rs[:], s[:])
                a = a_pool.tile([128, NCH, K], BF16)
                nc.vector.tensor_mul(
                    a[:], e[:], rs[:].unsqueeze(2).broadcast_to((128, NCH, K))
                )
                a_t[tok] = a

            def gemm2(tok):
                bi, ti = tok // TPB, tok % TPB
                g, hi = tok // GRP, tok % GRP
                if stage < 3:
                    if tok % GRP == GRP - 1:
                        dbg = o_pool.tile([65, GRP * K], BF16, name="o_t", tag="o_t")
                        nc.vector.tensor_copy(
                            dbg[:], a_t[tok][0:65, 0:GRP, 0:K]
                        )
                        nc.sync.dma_start(out=VO[g], in_=dbg[:])
                    a_t[tok] = None
                    return
                if hi == 0:
                    pv[g] = pv_pool.tile([65, GRP, K], F32, name="pv_t", tag="pv_t")
                for j in range(NCH):
                    nc.tensor.matmul(
                        pv[g][:, hi, :],
                        rn_sb[bi][:, ti, j, :],
                        a_t[tok][:, j, :],
                        start=(j == 0),
                        stop=(j == NCH - 1),
                        skip_group_check=True,
                    )
                a_t[tok] = None
                if hi == GRP - 1:
                    vo = o_pool.tile([65, GRP * K], BF16, name="o_t", tag="o_t")
                    nc.scalar.activation(
                        vo[:], pv[g][:], mybir.ActivationFunctionType.Copy
                    )
                    nc.sync.dma_start(out=VO[g], in_=vo[:])

            load_batch(0)
            for tok in range(TOK + LAG):
                if tok < TOK:
                    bi, ti = tok // TPB, tok % TPB
                    if ti == 0 and bi + 1 < NB:
                        load_batch(bi + 1)
                    gemm1_softmax(tok)
                lag_tok = tok - LAG
                if lag_tok >= 0:
                    gemm2(lag_tok)
    if split_waits:
        _split_excess_waits(nc)
    return nc


def _prep_core_inputs(r_core, WTB_h):
    """r_core: [TOK, N, C] fp32 -> per-core input map."""
    bf = ml_dtypes.bfloat16
    # RT: [NB, 65, TPB, N]; RT[b, c, t, n] = r[4b+t, n, c], row 64 = 1.0
    r5 = r_core.reshape(NB, TPB, N, C)                   # [b, t, n, c]
    rt = np.empty((NB, 65, TPB, N), dtype=np.float32)
    rt[:, :C] = r5.transpose(0, 3, 1, 2)
    rt[:, C] = 1.0
    # RN: [NB, 128, TPB, NCH, 65]; RN[b,p,t,j,:C] = r[4b+t, 128j+p, :], col 64 = -1
    r6 = r_core.reshape(NB, TPB, NCH, 128, C)            # [b, t, j, p, c]
    rn = np.empty((NB, 128, TPB, NCH, C + 1), dtype=np.float32)
    rn[..., :C] = r6.transpose(0, 3, 1, 2, 4)
    rn[..., C] = -1.0
    return {
        "RT": np.ascontiguousarray(rt.astype(bf)),
        "RN": np.ascontiguousarray(rn.astype(bf)),
        "WTB": WTB_h,
    }


def kernel(R_seq, W, b, centroids):
    if "nc" not in _CACHE:
        _CACHE["nc"] = _build_nc()
    nc = _CACHE["nc"]

    bf = ml_dtypes.bfloat16
    WTB_h = np.empty((65, K), dtype=np.float32)
    WTB_h[:C] = W.astype(np.float32).T
    WTB_h[C] = b.astype(np.float32)
    WTB_h = np.ascontiguousarray(WTB_h.astype(bf))

    r_all = np.asarray(R_seq, np.float32).reshape(NCORES, TOK, N, C)
    in_maps = [_prep_core_inputs(r_all[i], WTB_h) for i in range(NCORES)]

    res = run_bass_kernel_spmd(
        nc,
        in_maps,
        list(range(NCORES)),
        trace=bool(int(os.environ.get("NETVLAD_TRACE", "0"))),
    )
    _CACHE["last_results"] = res

    cent = np.asarray(centroids, np.float32)             # [K, C]
    outs = []
    for i in range(NCORES):
        vo = np.asarray(res.results[i]["VO"], np.float32)  # [NG, 65, GRP*K]
        vo = vo.reshape(NG, 65, GRP, K)
        vraw = vo[:, :C].transpose(0, 2, 3, 1).reshape(TOK, K, C)
        nasum = vo[:, C].transpose(0, 1, 2).reshape(TOK, K)  # = -sum_n a
        v = vraw + nasum[:, :, None] * cent[None]
        outs.append(v)
    out = np.stack(outs, axis=0).reshape(B, T, K, C).astype(np.float32)
    return out


if __name__ == "__main__":
    rng = np.random.default_rng(0)
    R = rng.normal(size=(B, T, N, C)).astype(np.float32)
    W_ = rng.normal(size=(K, C)).astype(np.float32) / 8.0
    b_ = (rng.normal(size=(K,)) * 0.01).astype(np.float32)
    cc = rng.normal(size=(K, C)).astype(np.float32)
    out = kernel(R, W_, b_, cc)
    print(out.shape, out.dtype)


# revision 7
# speedup vs baseline: 1.0246x; 1.0246x over previous
"""NetVLAD pooling kernel for Trainium2 (8 NeuronCores, data-parallel over B).

Math per token m (of B*T=256):  logits = r @ W.T + b ; a = softmax(logits, -1)
    v = a.T @ r - a.sum(0)[:, None] * centroids          (r: [N=2048, C=64], K=32)

Mapping (per core = 32 tokens):
  - GEMM1 (contract C, bias fused): lhsT = rT tiles [65, 128] where row 64 is
    constant 1.0; rhs = [W.T; b] [65, 32]. One matmul per 128-n chunk writes
    logits+b into one PSUM bank [128, 16, 32] per token.
  - softmax: one EXP (Scalar, PSUM f32 -> SBUF bf16), tensor_reduce over k
    (Vector), reciprocal, one bf16 multiply -> a [128, 16, 32] bf16.
  - GEMM2 (contract N, flipped): lhsT = rn chunk [128, 65] (col 64 = -1),
    rhs = a chunk [128, 32]. out = v.T [65, 32] per token, 16 tokens col-packed
    into one PSUM bank [65, 512]; row 64 = -sum_n(a).
  - No device epilogue: Scalar copies each vt bank to SBUF bf16, DMA to DRAM.
    Host applies v += (-asum) * centroids and transposes to [tok, K, C].
"""

import os
import sys

import numpy as np

sys.path.insert(0, "/opt/trn_rl_repo")

import ml_dtypes  # noqa: E402

import concourse.bass as bass  # noqa: E402
import concourse.tile as tile  # noqa: E402
from concourse import mybir  # noqa: E402
from concourse.bass_utils import run_bass_kernel_spmd  # noqa: E402

B, T, N, C, K = 8, 32, 2048, 64, 32
NCORES = 8
TOK = (B * T) // NCORES  # 32 tokens per core
TPB = 4                  # tokens per DMA batch
NB = TOK // TPB          # 8 batches
NCH = N // 128           # 16 n-chunks per token
GRP = 16                 # tokens per vt PSUM bank
NG = TOK // GRP          # 2 groups
LAG = 3                  # GEMM2 trails GEMM1 by LAG tokens (hides softmax)

BF16 = mybir.dt.bfloat16
F32 = mybir.dt.float32

_CACHE = {}

_NO_SPLIT_TYPES = ("InstEventSemaphore",)


def _split_excess_waits(nc):
    """walrus' setupSyncWait refuses >1 sem wait on (at least) the TT-family
    structs. Hoist extra waits onto standalone InstEventSemaphore ops."""
    for f in nc.m.functions:
        for blk in f.blocks:
            out = []
            changed = False
            for inst in blk.instructions:
                si = getattr(inst, "sync_info", None)
                if (
                    si is not None
                    and si.on_wait
                    and len(si.on_wait) > 1
                    and type(inst).__name__ not in _NO_SPLIT_TYPES
                ):
                    for idx, w in enumerate(si.on_wait[:-1]):
                        out.append(
                            mybir.InstEventSemaphore(
                                name=f"{inst.name}_xw{idx}",
                                engine=inst.engine,
                                sync_info=mybir.SyncInfo(on_wait=[w], on_update=[]),
                            )
                        )
                    inst.sync_info = mybir.SyncInfo(
                        on_wait=[si.on_wait[-1]], on_update=si.on_update
                    )
                    changed = True
                out.append(inst)
            if changed:
                try:
                    blk.instructions[:] = out
                except TypeError:
                    blk.instructions = out


def _build_nc(split_waits=True):
    stage = int(os.environ.get("NETVLAD_STAGE", "3"))  # 1=G1+exp 2=+softmax 3=full
    amul_eng = os.environ.get("NETVLAD_AMUL", "gpsimd")  # gpsimd|vector
    nc = bass.Bass()
    # RT split into two n-halves so per-partition DMA runs are 8KB (the
    # per-queue DMA rate drops ~40% for 16KB descriptors).
    RT = nc.declare_dram_parameter("RT", [NB, 65, 2, TPB, N // 2], BF16, False)
    RN = nc.declare_dram_parameter("RN", [NB, 128, TPB, NCH, 65], BF16, False)
    WTB = nc.declare_dram_parameter("WTB", [65, K], BF16, False)
    VO = nc.declare_dram_parameter("VO", [NG, 65, GRP * K], BF16, True)

    with tile.TileContext(nc) as tc:
        with (
            tc.tile_pool(name="singles", bufs=1) as singles,
            tc.tile_pool(name="rt", bufs=3) as rt_pool,
            tc.tile_pool(name="rn", bufs=3) as rn_pool,
            tc.tile_pool(name="e", bufs=3) as e_pool,
            tc.tile_pool(name="a", bufs=LAG + 3) as a_pool,
            tc.tile_pool(name="s", bufs=6) as s_pool,
            tc.tile_pool(name="o", bufs=2) as o_pool,
            tc.tile_pool(name="pl", bufs=3, space="PSUM") as pl_pool,
            tc.tile_pool(name="pv", bufs=2, space="PSUM") as pv_pool,
        ):
            wtb_sb = singles.tile([65, K], BF16)
            nc.sync.dma_start(out=wtb_sb[:], in_=WTB[:])

            rt_sb = [None] * NB
            rn_sb = [None] * NB
            pv = [None] * NG
            a_t = [None] * TOK

            def load_batch(bi):
                rt_sb[bi] = rt_pool.tile(
                    [65, 2, TPB, N // 2], BF16, name="rt_t", tag="rt_t"
                )
                nc.sync.dma_start(out=rt_sb[bi][:, 0], in_=RT[bi, :, 0])
                nc.sync.dma_start(out=rt_sb[bi][:, 1], in_=RT[bi, :, 1])
                rn_sb[bi] = rn_pool.tile(
                    [128, TPB, NCH, 65], BF16, name="rn_t", tag="rn_t"
                )
                nc.sync.dma_start(out=rn_sb[bi][:], in_=RN[bi])

            def gemm1_softmax(tok):
                bi, ti = tok // TPB, tok % TPB
                pl = pl_pool.tile([128, NCH, K], F32)
                for j in range(NCH):
                    h, jj = j // (NCH // 2), j % (NCH // 2)
                    nc.tensor.matmul(
                        pl[:, j, :],
                        rt_sb[bi][0:65, h, ti, 128 * jj : 128 * jj + 128],
                        wtb_sb[0:65, :],
                        start=True,
                        stop=True,
                        skip_group_check=True,
                    )
                e = e_pool.tile([128, NCH, K], BF16)
                nc.scalar.activation(
                    e[:], pl[:], mybir.ActivationFunctionType.Exp
                )
                if stage <= 1:
                    a_t[tok] = e
                    return
                s = s_pool.tile([128, NCH], F32)
                nc.vector.tensor_reduce(
                    s[:], e[:], axis=mybir.AxisListType.X, op=mybir.AluOpType.add
                )
                rs = s_pool.tile([128, NCH], F32)
                nc.vector.reciprocal(rs[:], s[:])
                a = a_pool.tile([128, NCH, K], BF16)
                eng = nc.gpsimd if amul_eng == "gpsimd" else nc.vector
                eng.tensor_mul(
                    a[:], e[:], rs[:].unsqueeze(2).broadcast_to((128, NCH, K))
                )
                a_t[tok] = a

            def gemm2(tok):
                bi, ti = tok // TPB, tok % TPB
                g, hi = tok // GRP, tok % GRP
                if stage < 3:
                    if tok % GRP == GRP - 1:
                        dbg = o_pool.tile([65, GRP * K], BF16, name="o_t", tag="o_t")
                        nc.vector.tensor_copy(
                            dbg[:], a_t[tok][0:65, 0:GRP, 0:K]
                        )
                        nc.sync.dma_start(out=VO[g], in_=dbg[:])
                    a_t[tok] = None
                    return
                if hi == 0:
                    pv[g] = pv_pool.tile([65, GRP, K], F32, name="pv_t", tag="pv_t")
                for j in range(NCH):
                    nc.tensor.matmul(
                        pv[g][:, hi, :],
                        rn_sb[bi][:, ti, j, :],
                        a_t[tok][:, j, :],
                        start=(j == 0),
                        stop=(j == NCH - 1),
                        skip_group_check=True,
                    )
                a_t[tok] = None
                if hi == GRP - 1:
                    vo = o_pool.tile([65, GRP * K], BF16, name="o_t", tag="o_t")
                    nc.scalar.activation(
                        vo[:], pv[g][:], mybir.ActivationFunctionType.Copy
                    )
                    nc.sync.dma_start(out=VO[g], in_=vo[:])

            load_batch(0)
            for tok in range(TOK + LAG):
                if tok < TOK:
                    bi, ti = tok // TPB, tok % TPB
                    if ti == 0 and bi + 1 < NB:
                        load_batch(bi + 1)
                    gemm1_softmax(tok)
                lag_tok = tok - LAG
                if lag_tok >= 0:
                    gemm2(lag_tok)
    if split_waits:
        _split_excess_waits(nc)
    return nc


def _prep_core_inputs(r_core, WTB_h):
    """r_core: [TOK, N, C] fp32 -> per-core input map."""
    bf = ml_dtypes.bfloat16
    # RT: [NB, 65, 2, TPB, N//2]; RT[b, c, h, t, nn] = r[4b+t, 1024h+nn, c],
    # row 64 = 1.0
    r5 = r_core.reshape(NB, TPB, 2, N // 2, C)           # [b, t, h, nn, c]
    rt = np.empty((NB, 65, 2, TPB, N // 2), dtype=np.float32)
    rt[:, :C] = r5.transpose(0, 4, 2, 1, 3)
    rt[:, C] = 1.0
    # RN: [NB, 128, TPB, NCH, 65]; RN[b,p,t,j,:C] = r[4b+t, 128j+p, :], col 64 = -1
    r6 = r_core.reshape(NB, TPB, NCH, 128, C)            # [b, t, j, p, c]
    rn = np.empty((NB, 128, TPB, NCH, C + 1), dtype=np.float32)
    rn[..., :C] = r6.transpose(0, 3, 1, 2, 4)
    rn[..., C] = -1.0
    return {
        "RT": np.ascontiguousarray(rt.astype(bf)),
        "RN": np.ascontiguousarray(rn.astype(bf)),
        "WTB": WTB_h,
    }


def kernel(R_seq, W, b, centroids):
    if "nc" not in _CACHE:
        _CACHE["nc"] = _build_nc()
    nc = _CACHE["nc"]

    bf = ml_dtypes.bfloat16
    WTB_h = np.empty((65, K), dtype=np.float32)
    WTB_h[:C] = W.astype(np.float32).T
    WTB_h[C] = b.astype(np.float32)
    WTB_h = np.ascontiguousarray(WTB_h.astype(bf))

    r_all = np.asarray(R_seq, np.float32).reshape(NCORES, TOK, N, C)
    in_maps = [_prep_core_inputs(r_all[i], WTB_h) for i in range(NCORES)]

    res = run_bass_kernel_spmd(
        nc,
        in_maps,
        list(range(NCORES)),
        trace=bool(int(os.environ.get("NETVLAD_TRACE", "0"))),
    )
    _CACHE["last_results"] = res

    cent = np.asarray(centroids, np.float32)             # [K, C]
    outs = []
    for i in range(NCORES):
        vo = np.asarray(res.results[i]["VO"], np.float32)  # [NG, 65, GRP*K]
        vo = vo.reshape(NG, 65, GRP, K)
        vraw = vo[:, :C].transpose(0, 2, 3, 1).reshape(TOK, K, C)
        nasum = vo[:, C].transpose(0, 1, 2).reshape(TOK, K)  # = -sum_n a
        v = vraw + nasum[:, :, None] * cent[None]
        outs.append(v)
    out = np.stack(outs, axis=0).reshape(B, T, K, C).astype(np.float32)
    return out


if __name__ == "__main__":
    rng = np.random.default_rng(0)
    R = rng.normal(size=(B, T, N, C)).astype(np.float32)
    W_ = rng.normal(size=(K, C)).astype(np.float32) / 8.0
    b_ = (rng.normal(size=(K,)) * 0.01).astype(np.float32)
    cc = rng.normal(size=(K, C)).astype(np.float32)
    out = kernel(R, W_, b_, cc)
    print(out.shape, out.dtype)
